# revision 1
# baseline (speedup 1.0000x reference)
"""Trainium2 Bass kernel for 7x7 valid cross-correlation on a 4096x4096 image.

Strategy: shard output rows across 8 NeuronCores (512 rows/core). Each core
receives its input row-slab WITH the (kh-1)=6 halo rows already included, so
no device-side halo exchange is needed. On-core, the conv is computed on the
tensor engine as 7 PSUM-accumulating matmuls per output tile: for each kernel
column dx, a banded-Toeplitz stationary matrix B_dx[k, m] = w[k-m, dx]
contracts over 128 input rows while the moving operand is a column-shifted
view X[:, c0+dx : c0+dx+N] of the input slab already in SBUF.

Every matmul is the same verified shape (K=128, M=122, N=512, fp32r): the
last row/column tiles overlap their predecessors and only the fresh rows
are written out (fp32r gives wrong results for partial K or odd N).
"""

import numpy as np

H, W = 4096, 4096
KH, KW = 7, 7
N_CORES = 8
OH, OW = H - KH + 1, W - KW + 1          # 4090, 4090
RPC = H // N_CORES                        # 512 output rows per core (core 7: 506 valid)
IN_ROWS = RPC + KH - 1                    # 518 input rows per core
MT = 122                                  # output rows per row tile (128 - 6)
# (input/output row offset within slab, rows of outt to emit: [emit0, 122))
ROW_TILES = [(0, 0), (122, 0), (244, 0), (366, 0), (390, 98)]
CT_N = 512
# column tile output starts; last overlaps so every matmul has N=512
COL_STARTS = [0, 512, 1024, 1536, 2048, 2560, 3072, 3578]

# fp32r = relaxed-precision fp32 matmul (TF32-like): 1 cycle/row vs 4 for fp32.
USE_FP32R = True

_cache = {}


def _build_program(repeat=1):
    import concourse.bacc as bacc
    import concourse.mybir as mybir
    import concourse.tile as tile

    mm_dt = mybir.dt.float32r if USE_FP32R else mybir.dt.float32
    f32 = mybir.dt.float32

    nc = bacc.Bacc("TRN2", target_bir_lowering=False, debug=False,
                   num_devices=N_CORES)
    x = nc.dram_tensor("x", [IN_ROWS, W], mm_dt, kind="ExternalInput")
    bands = nc.dram_tensor("bands", [128, KW, MT], mm_dt, kind="ExternalInput")
    biasb = nc.dram_tensor("biasb", [128, 1], f32, kind="ExternalInput")
    y = nc.dram_tensor("y", [RPC, OW], f32, kind="ExternalOutput")

    with tile.TileContext(nc) as tc:
        with (
            tc.tile_pool(name="const", bufs=1) as cpool,
            tc.tile_pool(name="xs", bufs=3) as xpool,
            tc.tile_pool(name="out", bufs=2) as opool,
            tc.tile_pool(name="ps", bufs=8, space="PSUM") as pspool,
        ):
            bands_t = cpool.tile([128, KW, MT], mm_dt)
            nc.sync.dma_start(bands_t[:], bands[:])
            bias_t = cpool.tile([128, 1], f32)
            nc.sync.dma_start(bias_t[:], biasb[:])

            # Slabs are DMAed in column chunks: the first column tiles'
            # matmuls start before the whole 2MB slab lands, and chunks
            # pipeline better with compute than one monolithic DMA.
            first_chunks = [(0, 1030), (1024, 1030), (2048, 1030),
                            (3072, 1024)]

            for rep in range(repeat):
                for it, (r0, emit0) in enumerate(ROW_TILES):
                    xs = xpool.tile([128, W], mm_dt, tag="xs")
                    chunks = first_chunks
                    for cc0, cw in chunks:
                        # scalar-engine HWDGE ring: runs parallel to the
                        # const/output DMAs on the sync-engine ring
                        nc.scalar.dma_start(xs[:, cc0:cc0 + cw],
                                            x[r0:r0 + 128, cc0:cc0 + cw])
                    outt = opool.tile([128, OW], f32, tag="out")
                    for c0 in COL_STARTS:
                        ps = pspool.tile([128, CT_N], f32, tag="ps")
                        for dx in range(KW):
                            nc.tensor.matmul(
                                ps[:MT, :],
                                bands_t[:, dx, :],
                                xs[:, c0 + dx: c0 + dx + CT_N],
                                start=(dx == 0),
                                stop=(dx == KW - 1),
                            )
                        nc.vector.tensor_scalar_add(
                            outt[:MT, c0:c0 + CT_N], ps[:MT, :],
                            bias_t[:MT, 0:1])
                    nc.sync.dma_start(
                        y[r0 + emit0: r0 + MT, :], outt[emit0:MT, :])

    nc.compile()
    return nc


def _get_program():
    if "nc" not in _cache:
        _cache["nc"] = _build_program()
    return _cache["nc"]


def _shard_inputs(X, weight, bias):
    X = np.ascontiguousarray(np.asarray(X, dtype=np.float32))
    weight = np.asarray(weight, dtype=np.float32)
    bias = np.asarray(bias, dtype=np.float32)

    # Host-side sharding: per-core input slab with halo rows (zero-padded at
    # the bottom edge for the last core; those output rows are discarded).
    slabs = np.zeros((N_CORES, IN_ROWS, W), dtype=np.float32)
    for i in range(N_CORES):
        r0 = RPC * i
        r1 = min(r0 + IN_ROWS, H)
        slabs[i, : r1 - r0] = X[r0:r1]

    # Banded-Toeplitz stationary matrices: bands[k, dx, m] = w[k-m, dx].
    bands = np.zeros((128, KW, MT), dtype=np.float32)
    for dy in range(KH):
        for m in range(MT):
            bands[m + dy, :, m] = weight[dy, :]

    biasb = np.broadcast_to(bias.reshape(1, 1), (128, 1)).copy()

    return [{"x": slabs[i], "bands": bands, "biasb": biasb}
            for i in range(N_CORES)]


def kernel(X, weight, bias):
    from concourse.bass_utils import run_bass_kernel_spmd

    nc = _get_program()
    in_maps = _shard_inputs(X, weight, bias)
    res = run_bass_kernel_spmd(nc, in_maps, list(range(N_CORES)))

    out = np.empty((OH, OW), dtype=np.float32)
    for i in range(N_CORES):
        r0 = RPC * i
        nrows = min(RPC, OH - r0)
        out[r0:r0 + nrows] = res.results[i]["y"][:nrows]
    return out



# revision 3
# speedup vs baseline: 2.0760x; 2.0760x over previous
"""Trainium2 Bass kernel for 7x7 valid cross-correlation on a 4096x4096 image.

Strategy: balanced spatial sharding across 8 NeuronCores. The tensor engine
computes the conv as 7 PSUM-accumulating matmuls per 122x512 output tile:
for each kernel column dx, a banded-Toeplitz stationary matrix
B_dx[k, m] = w[k-m, dx] contracts over 128 input rows while the moving
operand is a column-shifted view X[:, c0+dx : c0+dx+512] of the input slab
in SBUF. A matmul pass costs N=512 PE cycles regardless of K/M, so the
whole-image cost is driven purely by the number of 122-row x 512-col tile
passes; the sharding below makes every core run the same, minimal count:

  - rows 0..3903:   core i owns rows [488*i, 488*i+488) = exactly 4 full
                    122-row tiles (no partial-tile waste), all 4090 cols.
  - rows 3904..4089: the 186-row remainder (one 122-row + one 64-row tile)
                    is split column-wise: core i computes its 512-col unit
                    of both tiles. This strip is first in program order; its
                    small input DMA gets the PE started early.

All matmul operands are bf16 (1 cycle/row, half the HBM traffic of fp32);
PSUM accumulation stays fp32. Output is written back as bf16 and upcast on
the host (end-to-end error ~1e-3 relative, inside the 2e-2 gate). Outputs
are DMAed out in ~1024-col chunks as soon as the vector engine evacuates
them from PSUM, keeping the post-compute tail short.
"""

import numpy as np

H, W = 4096, 4096
KH, KW = 7, 7
N_CORES = 8
OH, OW = H - KH + 1, W - KW + 1          # 4090, 4090
MT = 122                                  # output rows per full row tile
CT_N = 512
# column tile output starts; last overlaps so every matmul has N=512
COL_STARTS = [0, 512, 1024, 1536, 2048, 2560, 3072, 3578]
# ya output DMA chunk boundaries (flushed after col tiles 1,3,5,7)
OUT_CHUNKS = [(1, 0, 1024), (3, 1024, 2048), (5, 2048, 3072), (7, 3072, OW)]

A_ROWS = 4 * MT                           # 488 output rows per core, block A
A_IN_ROWS = A_ROWS + KH - 1               # 494 input rows
B_ROW0 = N_CORES * A_ROWS                 # 3904: remainder strip start
B_ROWS = OH - B_ROW0                      # 186 = 122 + 64
B2_M = B_ROWS - MT                        # 64 rows in the second strip tile
B_IN_ROWS = H - B_ROW0                    # 192 input rows for the strip
B_IN_COLS = CT_N + KH - 1                 # 518

_cache = {}


def _build_program(repeat=1):
    import concourse.bacc as bacc
    import concourse.mybir as mybir
    import concourse.tile as tile

    mm_dt = mybir.dt.bfloat16
    f32 = mybir.dt.float32

    nc = bacc.Bacc("TRN2", target_bir_lowering=False, debug=False,
                   num_devices=N_CORES)
    xa = nc.dram_tensor("xa", [A_IN_ROWS, W], mm_dt, kind="ExternalInput")
    xb = nc.dram_tensor("xb", [B_IN_ROWS, B_IN_COLS], mm_dt,
                        kind="ExternalInput")
    bands = nc.dram_tensor("bands", [128, KW, MT], mm_dt, kind="ExternalInput")
    biasb = nc.dram_tensor("biasb", [128, 1], f32, kind="ExternalInput")
    ya = nc.dram_tensor("ya", [A_ROWS, OW], mm_dt, kind="ExternalOutput")
    yb = nc.dram_tensor("yb", [B_ROWS, CT_N], mm_dt, kind="ExternalOutput")

    with tile.TileContext(nc) as tc:
        with (
            tc.tile_pool(name="const", bufs=1) as cpool,
            tc.tile_pool(name="xs", bufs=3) as xpool,
            tc.tile_pool(name="out", bufs=2) as opool,
            tc.tile_pool(name="ps", bufs=8, space="PSUM") as pspool,
        ):
            bands_t = cpool.tile([128, KW, MT], mm_dt)
            nc.sync.dma_start(bands_t[:], bands[:])
            bias_t = cpool.tile([128, 1], f32)
            nc.sync.dma_start(bias_t[:], biasb[:])

            # xa slabs are DMAed per row tile in column chunks so matmuls on
            # the first columns start before the whole slab lands.
            xa_chunks = [(0, 1030), (1024, 1030), (2048, 1030), (3072, 1024)]

            for rep in range(repeat):
                # --- remainder strip first: tiny DMA, starts the PE early ---
                xb1 = xpool.tile([128, B_IN_COLS], mm_dt, tag="xb1")
                nc.scalar.dma_start(xb1[:], xb[0:128, :])
                xb2 = xpool.tile([B2_M + KH - 1, B_IN_COLS], mm_dt, tag="xb2")
                nc.scalar.dma_start(xb2[:], xb[MT:B_IN_ROWS, :])

                outb = opool.tile([128, CT_N], mm_dt, tag="outb")
                ps = pspool.tile([128, CT_N], f32, tag="ps")
                for dx in range(KW):
                    nc.tensor.matmul(ps[:MT, :], bands_t[:, dx, :],
                                     xb1[:, dx:dx + CT_N],
                                     start=(dx == 0), stop=(dx == KW - 1))
                nc.vector.tensor_scalar_add(outb[:MT, :], ps[:MT, :],
                                            bias_t[:MT, 0:1])
                nc.sync.dma_start(yb[0:MT, :], outb[:MT, :])

                outb2 = opool.tile([B2_M, CT_N], mm_dt, tag="outb2")
                ps = pspool.tile([128, CT_N], f32, tag="ps")
                for dx in range(KW):
                    nc.tensor.matmul(ps[:B2_M, :], bands_t[:B2_M + KH - 1, dx, :B2_M],
                                     xb2[:, dx:dx + CT_N],
                                     start=(dx == 0), stop=(dx == KW - 1))
                nc.vector.tensor_scalar_add(outb2[:], ps[:B2_M, :],
                                            bias_t[:B2_M, 0:1])
                nc.sync.dma_start(yb[MT:B_ROWS, :], outb2[:])

                # --- block A: 4 full 122-row tiles over all 4090 columns ---
                for it in range(4):
                    r0 = it * MT
                    xs = xpool.tile([128, W], mm_dt, tag="xs")
                    for cc0, cw in xa_chunks:
                        nc.scalar.dma_start(xs[:, cc0:cc0 + cw],
                                            xa[r0:r0 + 128, cc0:cc0 + cw])
                    outt = opool.tile([128, OW], mm_dt, tag="out")
                    for ci, c0 in enumerate(COL_STARTS):
                        ps = pspool.tile([128, CT_N], f32, tag="ps")
                        for dx in range(KW):
                            nc.tensor.matmul(
                                ps[:MT, :],
                                bands_t[:, dx, :],
                                xs[:, c0 + dx: c0 + dx + CT_N],
                                start=(dx == 0),
                                stop=(dx == KW - 1),
                            )
                        nc.vector.tensor_scalar_add(
                            outt[:MT, c0:c0 + CT_N], ps[:MT, :],
                            bias_t[:MT, 0:1])
                        for fci, a, b in OUT_CHUNKS:
                            if fci == ci:
                                nc.sync.dma_start(ya[r0:r0 + MT, a:b],
                                                  outt[:MT, a:b])

    nc.compile()
    return nc


def _get_program():
    if "nc" not in _cache:
        _cache["nc"] = _build_program()
    return _cache["nc"]


def _shard_inputs(X, weight, bias):
    import ml_dtypes

    bf16 = ml_dtypes.bfloat16
    X = np.ascontiguousarray(np.asarray(X, dtype=np.float32)).astype(bf16)
    weight = np.asarray(weight, dtype=np.float32)
    bias = np.asarray(bias, dtype=np.float32)

    # Banded-Toeplitz stationary matrices: bands[k, dx, m] = w[k-m, dx].
    bands = np.zeros((128, KW, MT), dtype=np.float32)
    for dy in range(KH):
        for m in range(MT):
            bands[m + dy, :, m] = weight[dy, :]
    bands = bands.astype(bf16)

    biasb = np.broadcast_to(bias.reshape(1, 1), (128, 1)).copy()

    in_maps = []
    for i in range(N_CORES):
        r0 = A_ROWS * i
        cs = COL_STARTS[i]
        in_maps.append({
            "xa": np.ascontiguousarray(X[r0:r0 + A_IN_ROWS]),
            "xb": np.ascontiguousarray(X[B_ROW0:, cs:cs + B_IN_COLS]),
            "bands": bands,
            "biasb": biasb,
        })
    return in_maps


def kernel(X, weight, bias):
    from concourse.bass_utils import run_bass_kernel_spmd

    nc = _get_program()
    in_maps = _shard_inputs(X, weight, bias)
    res = run_bass_kernel_spmd(nc, in_maps, list(range(N_CORES)))

    out = np.empty((OH, OW), dtype=np.float32)
    for i in range(N_CORES):
        r0 = A_ROWS * i
        out[r0:r0 + A_ROWS] = res.results[i]["ya"].astype(np.float32)
        cs = COL_STARTS[i]
        out[B_ROW0:, cs:cs + CT_N] = res.results[i]["yb"].astype(np.float32)
    return out
